# revision 79
# baseline (speedup 1.0000x reference)
"""Causal self-attention kernel for 8 TRN2 NeuronCores.

Problem (hardcoded): B=2, T=4096, C=768, NH=12, HS=64.
  qkv = x @ w_attn + b_attn; per-head causal softmax attention;
  y = att_out @ w_proj + b_proj

Sharding: 24 (batch, head) units over 8 cores -> 3 heads per core.
  cores 0..3: batch 0, heads (0,1,2), (3,4,5), (6,7,8), (9,10,11)
  cores 4..7: batch 1, same head split.
Each core computes a partial y^T [C, T]; the host sums partials per batch
and adds b_proj (plus the folded v-bias term b_v @ w_proj, which is exact
because softmax rows sum to 1). The host pre-transposes x and converts
weights/activations to bf16.

Datapath (all matmuls bf16 -> fp32 PSUM; no 2-pass fp32 matmuls):
 - Q/K computed as [d, t] via W^T x^T; V computed directly in [keys, d]
   layout via the flipped product (x^T chunk stationary, W_v moving), so
   no PE transposes are needed. The ones-column of V'' that produces the
   softmax denominators is pre-set once by memset.
 - Exact-causal narrowing: diagonal k-tiles compute only the valid query
   columns, with the two halves of a unit packed contiguously
   ([m, 512) and [512, 1024-m)) so a single ScalarE exp covers both.

Schedule: the steady state is ScalarE(exp)-bound (~1.1us/unit vs
~0.65us/unit of S+AV on the PE), so all other PE work is interleaved
into the unit loops as thunks: t-block tb's normalize+projection runs
inside tb+1's unit loop (its reciprocals run on DVE during tb's C phase
/ loop tail), and tb+1's QKV + V matmuls run in the tail of tb's loop.
The PE therefore never idles long enough for the HAM clock gate to
re-throttle it to 1.2 GHz.
"""

import numpy as np

B, T, C, NH = 2, 4096, 768, 12
HS = C // NH          # 64
NCORES = 8
HPC = 3               # heads per core
QB = 512              # q block (moving dim)
NKT = T // 128        # 32 k-tiles
NTB = T // QB         # 8 t-blocks
NCC = C // 128        # 6 contraction chunks
VP_W = 65             # V'' width per k-tile (64 v dims + ones col)
WQJ = 4 * 128         # q/k j-blocks: [qA|qB] [kA|kB] [qC|qC] [kC|kC]

_CACHE = {}


def _build():
    import contextlib
    import concourse.bacc as bacc
    import concourse.mybir as mybir
    from concourse.tile import TileContext

    f32 = mybir.dt.float32
    f32r = mybir.dt.float32r
    bf16 = mybir.dt.bfloat16
    Exp = mybir.ActivationFunctionType.Exp
    Ident = mybir.ActivationFunctionType.Identity
    mult = mybir.AluOpType.mult

    nc = bacc.Bacc(trn_type="TRN2")

    xt_d = nc.dram_tensor("xt", [C, T], bf16, kind="ExternalInput")
    wq = nc.dram_tensor("wq", [C, WQJ], bf16, kind="ExternalInput")
    wv = nc.dram_tensor("wv", [C, HPC * HS], bf16, kind="ExternalInput")
    bq = nc.dram_tensor("bq", [128, 4], f32, kind="ExternalInput")
    wp = nc.dram_tensor("wp", [HPC * HS, C], bf16, kind="ExternalInput")
    y = nc.dram_tensor("y", [C, T], bf16, kind="ExternalOutput")

    with TileContext(nc) as tc, nc.allow_low_precision("bf16 kernel"):
        with contextlib.ExitStack() as ctx:
            cpool = ctx.enter_context(tc.tile_pool(name="const", bufs=1))
            keep = ctx.enter_context(tc.tile_pool(name="keep", bufs=1))
            xtp_p = ctx.enter_context(tc.tile_pool(name="xtp", bufs=2))
            pt_p = ctx.enter_context(tc.tile_pool(name="ptp", bufs=4))
            on_p = ctx.enter_context(tc.tile_pool(name="onp", bufs=2))
            ovs_p = ctx.enter_context(tc.tile_pool(name="ovsp", bufs=1))
            rr_p = ctx.enter_context(tc.tile_pool(name="rrp", bufs=1))
            ys_p = ctx.enter_context(tc.tile_pool(name="ysp", bufs=3))
            sps_p = ctx.enter_context(
                tc.tile_pool(name="sps", bufs=2, space="PSUM"))
            ov_p = ctx.enter_context(
                tc.tile_pool(name="ovp", bufs=1, space="PSUM"))
            sm_p = ctx.enter_context(
                tc.tile_pool(name="smp", bufs=2, space="PSUM"))

            mask = cpool.tile([128, 896], f32)
            nc.gpsimd.memset(mask[:], 1.0)
            nc.gpsimd.affine_select(
                out=mask[:], in_=mask[:], compare_op=mybir.AluOpType.is_ge,
                fill=0.0, base=-384, channel_multiplier=-1, pattern=[[1, 896]])
            mask_b = cpool.tile([128, 896], bf16)
            nc.vector.tensor_copy(mask_b[:], mask[:])
            ones_t = cpool.tile([128, 64], f32)
            nc.gpsimd.memset(ones_t[:], 1.0)
            ones_r = cpool.tile([128, 64], f32r)
            nc.vector.tensor_copy(ones_r[:], ones_t[:])
            ones_b = cpool.tile([128, 64], bf16)
            nc.gpsimd.memset(ones_b[:], 1.0)

            bq_sb = cpool.tile([128, 4], f32)
            nc.sync.dma_start(bq_sb[:], bq[:, :])
            wp_sb = [keep.tile([64, C], bf16, tag=f"wp{h}", name=f"wp{h}")
                     for h in range(HPC)]

            QT_AB = keep.tile([128, T], bf16, tag="qt_ab")
            KT_AB = keep.tile([128, T], bf16, tag="kt_ab")
            QT_C = keep.tile([128, T], bf16, tag="qt_c")
            KT_C = keep.tile([128, T], bf16, tag="kt_c")
            QKT = (QT_AB, KT_AB, QT_C, KT_C)
            # V'' for all heads: [keys%128, head, k-tile, 64 v + ones col]
            Vp = keep.tile([128, HPC, NKT, VP_W], bf16, tag="vp")
            nc.gpsimd.memset(Vp[:, :, :, 64:65], 1.0)

            wq_sb = [keep.tile([128, WQJ], bf16, tag=f"wq{cc}",
                               name=f"wq{cc}") for cc in range(NCC)]
            wv_sb = [keep.tile([128, HPC * HS], bf16, tag=f"wv{cc}",
                               name=f"wv{cc}") for cc in range(NCC)]

            xt_tiles = {}
            # reciprocal chunks ([1,64] DVE ops) pending issue; drained a
            # couple per unit so they interleave with other DVE work
            # instead of blocking the strict-FIFO DVE queue for 3+us
            rq = []

            def issue_xt_dma(t, with_w=False):
                t0 = t * QB
                xt = xtp_p.tile([128, NCC, QB], bf16, tag="xt",
                                name=f"xt{t}")
                xt_tiles[t] = xt
                for cc in range(NCC):
                    nc.gpsimd.dma_start(
                        xt[:, cc, :],
                        xt_d[cc * 128:(cc + 1) * 128, t0:t0 + QB])
                    if with_w:
                        nc.gpsimd.dma_start(
                            wq_sb[cc][:], wq[cc * 128:(cc + 1) * 128, :])
                if with_w:
                    # after xt+wq so the first QKV matmul starts sooner
                    for cc in range(NCC):
                        nc.gpsimd.dma_start(
                            wv_sb[cc][:], wv[cc * 128:(cc + 1) * 128, :])

            qp_tiles = {}

            def qkv_half(t, blk, half):
                """Half a QKV block (3 contraction chunks) so the thunk
                fits the per-unit PE slack without bubbling the exp
                stream; the PSUM accumulation group stays open across
                interleaved matmuls to other banks."""
                xt = xt_tiles[t]
                if half == 0:
                    qp_tiles[(t, blk)] = sm_p.tile(
                        [128, QB], f32, tag="small", name=f"qp{t}_{blk}")
                qp = qp_tiles[(t, blk)]
                for cc in range(3 * half, 3 * half + 3):
                    nc.tensor.matmul(
                        qp[:], wq_sb[cc][:, blk * 128:(blk + 1) * 128],
                        xt[:, cc, :], start=(cc == 0), stop=(cc == NCC - 1),
                        skip_group_check=True)
                if half == 1:
                    qkv_bias(t, blk, qp_tiles.pop((t, blk)))

            def qkv_blk(t, blk):
                qkv_half(t, blk, 0)
                qkv_half(t, blk, 1)

            def qkv_bias(t, blk, qp):
                t0 = t * QB
                if t <= 3:
                    # early t-blocks are DVE-bound; ScalarE idles there
                    nc.scalar.activation(
                        QKT[blk][:, t0:t0 + QB], qp[:], Ident,
                        bias=bq_sb[:, blk:blk + 1])
                else:
                    nc.vector.tensor_scalar_add(
                        QKT[blk][:, t0:t0 + QB], qp[:],
                        bq_sb[:, blk:blk + 1])

            vps_tiles = {}

            def v_half(t, ts, half):
                xt = xt_tiles[t]
                kt = 4 * t + ts
                if half == 0:
                    vps_tiles[kt] = sm_p.tile(
                        [128, HPC * HS], f32, tag="small", name=f"v{t}_{ts}")
                vps = vps_tiles[kt]
                for cc in range(3 * half, 3 * half + 3):
                    nc.tensor.matmul(
                        vps[:], xt[:, cc, ts * 128:(ts + 1) * 128],
                        wv_sb[cc][:], start=(cc == 0), stop=(cc == NCC - 1),
                        skip_group_check=True)
                if half == 1:
                    del vps_tiles[kt]
                    nc.vector.tensor_copy(
                        Vp[:, :, kt, 0:HS],
                        vps[:].rearrange("p (h d) -> p h d", d=HS))

            def v_blk(t, ts):
                v_half(t, ts, 0)
                v_half(t, ts, 1)

            def attention(tb, carry, flush_prev, with_next):
                """Returns (carry, flush): normalize+projection thunks and
                the final AV-flush closure for this t-block, both run
                inside the next t-block's unit loop (the flush after two
                S-issues, so the exp stream has no boundary bubble)."""
                q0 = tb * QB
                nkt = 4 * tb + 4
                units = ([("AB", kt) for kt in range(nkt)]
                         + [("C", s) for s in range(nkt // 2)])

                if with_next:
                    issue_xt_dma(tb + 1)
                    late = ([lambda blk=b, hf=h: qkv_half(tb + 1, blk, hf)
                             for b in range(4) for h in range(2)]
                            + [lambda ts=t, hf=h: v_half(tb + 1, ts, hf)
                               for t in range(4) for h in range(2)])
                else:
                    late = []
                carry = list(carry)

                ovA = ov_p.tile([128, QB], f32, tag="ovA", name=f"ovA{tb}")
                ovB = ov_p.tile([128, QB], f32, tag="ovB", name=f"ovB{tb}")
                ov_of = {0: ovA, 1: ovB}
                # evacuated O (all heads) + 1/denom via exp(-ln d) on ACT
                ovs3 = ovs_p.tile([VP_W, HPC, QB], f32, tag="ovs",
                                  name=f"ovs{tb}")
                # (ScalarE exp(-ln d) reciprocals were tried for the
                # DVE-bound early t-blocks, but the ACT table reloads
                # (~1.3us each, Exp<->Ln) cost more than they saved)
                act_recip = False
                rr3 = rr_p.tile([VP_W, HPC, QB], f32r, tag="rr",
                                name=f"rr{tb}")

                def halves_of(u):
                    kind, k = u
                    kts = (k, k) if kind == "AB" else (2 * k, 2 * k + 1)
                    qt, kt_t = (QT_AB, KT_AB) if kind == "AB" \
                        else (QT_C, KT_C)
                    out = []
                    for half, kt in enumerate(kts):
                        m = kt * 128 - q0
                        out.append((kt, qt, kt_t, (64 * half, 64 * half + 64),
                                    max(m, 0), 0 <= m < QB))
                    return out

                def issue_S(u):
                    kind, k = u
                    sps = sps_p.tile([128, 1024], f32, tag="sps",
                                     name=f"s{kind}{tb}_{k}")
                    hv = halves_of(u)
                    for half, (kt, qt, kt_t, (r0, r1), m, _) in enumerate(hv):
                        dest = sps[:, m:QB] if half == 0 \
                            else sps[:, QB:2 * QB - m]
                        nc.tensor.matmul(
                            dest, kt_t[r0:r1, kt * 128:(kt + 1) * 128],
                            qt[r0:r1, q0 + m:q0 + QB], start=True, stop=True)
                    pt = pt_p.tile([128, 1024], bf16, tag="pt",
                                   name=f"pt{kind}{tb}_{k}")
                    m0, m1 = hv[0][4], hv[1][4]
                    nc.scalar.activation(
                        pt[:, m0:2 * QB - m1], sps[:, m0:2 * QB - m1],
                        Exp, scale=0.125)
                    return pt

                def issue_AV(u, pt):
                    kind, k = u
                    hv = halves_of(u)
                    # C-unit masks (and all early-block masks) run on
                    # gpsimd so they never queue behind the reciprocals
                    # on the DVE FIFO
                    meng = nc.vector if (kind == "AB" and nkt > 12) \
                        else nc.gpsimd
                    for half, (kt, _, _, _, m, diag) in enumerate(hv):
                        src = pt[:, m:QB] if half == 0 \
                            else pt[:, QB:2 * QB - m]
                        if diag:
                            meng.tensor_tensor(
                                out=src, in0=src,
                                in1=mask_b[:, 384:896 - m], op=mult)
                        if kind == "AB":
                            h, ov = half, ov_of[half]
                        else:
                            h, ov = 2, ov_of[2]
                        nc.tensor.matmul(
                            ov[0:VP_W, m:QB], Vp[:, h, kt, :], src,
                            start=(kt == 0), stop=(kt == nkt - 1))

                def evac(h):
                    nc.vector.tensor_copy(ovs3[:, h, :],
                                          ov_of[h][0:VP_W, :])
                    if act_recip:
                        return
                    for c in range(4):
                        rq.append((h, lambda h=h, c=c: nc.vector.reciprocal(
                            rr3[64:65, h, c * 128:(c + 1) * 128],
                            ovs3[64:65, h, c * 128:(c + 1) * 128])))

                # S runs TWO units ahead of AV so the ScalarE exp stream
                # never waits on an S matmul queued behind an AV (the exp
                # is the pacing engine; its input is always ready).
                pending = []

                def pop_av():
                    v = pending.pop(0)
                    issue_AV(*v)
                    if v[0] == ("AB", nkt - 1):
                        evac(0)
                        evac(1)

                for i, u in enumerate(units):
                    if u == ("C", 0):
                        # reuses ovA's PSUM bank; safe because ovA was
                        # evacuated to SBUF right after its last AV
                        ov_of[2] = ov_p.tile([128, QB], f32, tag="ovA",
                                             name=f"ovC{tb}")
                    pending.append((u, issue_S(u)))
                    if i == 1 and flush_prev:
                        flush_prev.pop()()
                    if len(pending) > 2:
                        pop_av()
                    for _ in range(2):
                        if rq:
                            rq.pop(0)[1]()
                    # strictly one thunk per unit: popping two bubbles the
                    # exp stream more than end-of-loop spills cost
                    if i >= 3 and carry:
                        carry.pop(0)()
                    elif i >= 6 and late:
                        late.pop(0)()

                def final_flush():
                    while pending:
                        pop_av()
                    evac(2)

                if with_next:
                    flush = [final_flush]
                else:
                    final_flush()
                    flush = []
                    while rq:
                        rq.pop(0)[1]()
                for t in carry:
                    t()
                for t in late:
                    t()

                # normalize + projection thunks, run inside the next loop
                ons = {}

                def rbp_thunk(h):
                    def f():
                        # all of head h's reciprocal chunks must be issued
                        # before the broadcast reads rr3[:, h, :]
                        while any(e[0] == h for e in rq):
                            rq.pop(0)[1]()
                        # broadcast 1/denom to 64 rows on the PE (gpsimd
                        # partition_broadcast passed CoreSim but produced
                        # wrong results on hardware)
                        rbp = sm_p.tile([64, QB], f32, tag="small",
                                        name=f"rbp{tb}_{h}")
                        nc.tensor.matmul(
                            rbp[:], ones_r[64:65, :], rr3[64:65, h, :],
                            start=True, stop=True)
                        on = on_p.tile([64, QB], bf16, tag=f"on{h}",
                                       name=f"on{tb}_{h}")
                        nc.vector.tensor_tensor(
                            out=on[:], in0=ovs3[0:64, h, :], in1=rbp[:],
                            op=mult)
                        ons[h] = on
                    return f

                yp_tiles = {}

                def proj_thunk(co, half):
                    def f():
                        if half == 0:
                            yp_tiles[co] = sm_p.tile(
                                [128, QB], f32, tag="small",
                                name=f"yp{tb}_{co}")
                        yp = yp_tiles[co]
                        hs = (0, 1) if half == 0 else (2,)
                        for h in hs:
                            nc.tensor.matmul(
                                yp[:], wp_sb[h][:, co * 128:(co + 1) * 128],
                                ons[h][:], start=(h == 0),
                                stop=(h == HPC - 1), skip_group_check=True)
                        if half == 0:
                            return
                        del yp_tiles[co]
                        ys = ys_p.tile([128, QB], bf16, tag="ys",
                                       name=f"ys{tb}_{co}")
                        nc.vector.tensor_copy(ys[:], yp[:])
                        nc.sync.dma_start(
                            y[co * 128:(co + 1) * 128, q0:q0 + QB], ys[:])
                    return f

                return ([rbp_thunk(h) for h in range(HPC)]
                        + [proj_thunk(co, hf)
                           for co in range(NCC) for hf in range(2)], flush)

            # prologue: t-block 0's QKV + V, then the pipelined loop
            issue_xt_dma(0, with_w=True)
            for h in range(HPC):
                nc.gpsimd.dma_start(wp_sb[h][:], wp[h * 64:(h + 1) * 64, :])
            for blk in range(4):
                qkv_blk(0, blk)
            for ts in range(4):
                v_blk(0, ts)
            carry, flush = [], []
            for tb in range(NTB):
                carry, flush = attention(tb, carry, flush,
                                         with_next=(tb + 1 < NTB))
            for t in carry:                        # final t-block epilogue
                t()

    nc.finalize()
    return nc


def _core_inputs(x, w_attn, b_attn, w_proj):
    """Build the 8 per-core input maps (bf16 weights/activations)."""
    import ml_dtypes
    bf = ml_dtypes.bfloat16
    maps = []
    for core in range(NCORES):
        b = core // 4
        heads = [HPC * (core % 4) + k for k in range(HPC)]
        hA, hB, hC = heads
        qc = lambda h: slice(h * HS, (h + 1) * HS)
        kc = lambda h: slice(C + h * HS, C + (h + 1) * HS)
        vc = lambda h: slice(2 * C + h * HS, 2 * C + (h + 1) * HS)
        wqm = np.concatenate([
            w_attn[:, qc(hA)], w_attn[:, qc(hB)],
            w_attn[:, kc(hA)], w_attn[:, kc(hB)],
            w_attn[:, qc(hC)], w_attn[:, qc(hC)],
            w_attn[:, kc(hC)], w_attn[:, kc(hC)],
        ], axis=1)
        wvm = np.concatenate([w_attn[:, vc(h)] for h in heads], axis=1)
        bqm = np.zeros((128, 4), np.float32)
        bqm[0:64, 0] = b_attn[qc(hA)]
        bqm[64:128, 0] = b_attn[qc(hB)]
        bqm[0:64, 1] = b_attn[kc(hA)]
        bqm[64:128, 1] = b_attn[kc(hB)]
        bqm[0:64, 2] = b_attn[qc(hC)]
        bqm[64:128, 2] = b_attn[qc(hC)]
        bqm[0:64, 3] = b_attn[kc(hC)]
        bqm[64:128, 3] = b_attn[kc(hC)]
        wpm = np.concatenate([w_proj[h * HS:(h + 1) * HS, :] for h in heads],
                             axis=0)
        maps.append({
            "xt": np.ascontiguousarray(x[b].T).astype(bf),
            "wq": np.ascontiguousarray(wqm).astype(bf),
            "wv": np.ascontiguousarray(wvm).astype(bf),
            "bq": bqm,
            "wp": np.ascontiguousarray(wpm).astype(bf),
        })
    return maps


def run_cores(in_maps, trace=False):
    from concourse import bass_utils
    if "nc" not in _CACHE:
        _CACHE["nc"] = _build()
    return bass_utils.run_bass_kernel_spmd(
        _CACHE["nc"], in_maps, list(range(NCORES)), trace=trace)


def kernel(x, w_attn, b_attn, w_proj, b_proj):
    x = np.asarray(x, np.float32)
    w_attn = np.asarray(w_attn, np.float32)
    b_attn = np.asarray(b_attn, np.float32)
    w_proj = np.asarray(w_proj, np.float32)
    b_proj = np.asarray(b_proj, np.float32)

    in_maps = _core_inputs(x, w_attn, b_attn, w_proj)
    res = run_cores(in_maps)
    # v-bias contributes exactly b_v @ w_proj per row (softmax rows sum
    # to 1), so it is folded in here on the host.
    bias_out = b_attn[2 * C:] @ w_proj + b_proj
    y = np.zeros((B, T, C), np.float32)
    for b in range(B):
        acc = np.zeros((C, T), np.float64)
        for core in range(4 * b, 4 * b + 4):
            acc += res.results[core]["y"].astype(np.float64)
        y[b] = acc.T + bias_out[None, :]
    return y
